# revision 42
# baseline (speedup 1.0000x reference)
"""Trainium2 Bass kernel for the CustomAttn module (causal attention + ALiBi).

Sharding: heads across 8 cores (4 heads/core).  W_attn is column-sharded into
per-head q/k/v blocks, W_proj row-sharded correspondingly; each core produces a
partial [S, E] output and the host sums the 8 partials (plus the bias folds).

Layout trick: everything is computed transposed (qk^T: [feat, S]; attn^T:
[feat, S]) so that no on-device transposes are needed anywhere:
  - qk^T tiles come from matmul(lhsT=W_slice, rhs=X^T)
  - V comes naturally from matmul(lhsT=X^T, rhs=Wv)
  - scores^T[c, r] from matmul(lhsT=k^T, rhs=q^T), softmax'd unnormalized via
    exp (ACT) x Toeplitz mask tile (ALiBi decay * causal; only ~5 of 16
    key-blocks per query-block survive the fp32 underflow of the ALiBi decay)
  - attn^T from matmul(lhsT=[V | ones], rhs=probs^T): the ones column gives the
    softmax denominator as psum row 64 for free
  - the projection uses attn^T directly as lhsT.
The 1/sqrt(D) score scale is folded into Wq on the host.

v3 structure: the QKV GEMM (phase A) runs in half-passes (2 of 4 m-blocks per
sweep) so its psum footprint drops to 6 banks, leaving a 2-bank ring for the
score matmuls of query-block qb-1, whose sc->exp->mask chains are spliced into
the GEMM's k-loop -- the GEMM's dense matmul stream hides their latency and
the scalar/vector engines' slack absorbs exp/mask.  Phase B is then a nearly
pure matmul stream (pv accumulation + reciprocal-broadcast + projection) with
per-head staggered chains and deferred normalize/projection closures spliced
as PE filler.  qk psum tags are double-buffered across half-passes so no
matmul waits on a drain at a group boundary.

All tensors are bf16 on the wire and in SBUF (psum accumulation fp32);
the host sums the 8 bf16 partials in fp32.
"""

import sys
from contextlib import ExitStack

if "/opt/trn_rl_repo" not in sys.path:
    sys.path.insert(0, "/opt/trn_rl_repo")

import numpy as np

S = 2048
E = 2048
D = 64
HLOC = 4          # heads per core
N_CORES = 8
P = 128
NG = S // 512     # 4 column groups of 512
KE = E // 128     # 16 contraction tiles
ALIBI_M = 2.0 ** (-0.25)
NMASK = 5         # mask tiles for didx = kb - 4*qb + 1 in 0..4


def _install_drain_patch():
    """This walrus build rejects a multi-wait SP Drain at the Tile kernel tail
    ("Too many sync wait commands"); split the waits into standalone
    EventSemaphore waits ahead of a bare drain."""
    from concourse import tile as _tile
    from concourse.vector_clock import ScopedClock

    if getattr(_tile.TileContext, "_drain_patch_installed", False):
        return

    def _patched(self, tick_clock, wait_clock):
        nc = self.nc
        probe = nc.sync.nop()
        wait_clock.add_sem_waits(
            probe.ins, ScopedClock({None: tick_clock.global_clock})
        )
        waits = list(probe.ins.sync_info.on_wait) if probe.ins.sync_info else []
        probe.ins.sync_info = None
        sems_by_name = {s.name: s for s in self.sems.allocated().values()}
        for w in waits:
            nc.sync.wait_ge(sems_by_name[w.ant_name], w.wait_value)
        nc.sync.drain()
        nc.all_engine_barrier()
        popped = nc._tile_sem_poison_stack.pop()
        assert popped is self._sem_poison
        nc.clear_and_free_semaphores(list(self.sems.allocated().values()))
        nc.all_engine_barrier()

    _tile.TileContext._drain_and_barrier = _patched
    _tile.TileContext._drain_patch_installed = True


MAX_WAITS = 1
MAX_WAITS_BY_OP = {"DMACopy": 1, "DMATranspose": 1, "Drain": 1, "NoOp": 1}


def _install_wait_split_patch():
    """Same walrus limitation, general form: instructions with more than
    MAX_WAITS sem-waits fail codegen ("Too many sync wait commands").  Hoist
    the excess waits onto standalone EventSemaphore instructions immediately
    before the instruction on the same engine queue (in-order execution makes
    that equivalent gating)."""
    from concourse import tile as _tile
    from concourse import mybir

    if getattr(_tile.TileContext, "_wait_split_installed", False):
        return
    orig_add = _tile.TileContext._add_instruction

    def _patched_add(self, inst):
        si = inst.sync_info
        lim = MAX_WAITS_BY_OP.get(type(inst).__name__.replace("Inst", ""), MAX_WAITS)
        try:
            opname = inst.concise_opcode()
        except Exception:
            opname = ""
        if opname in MAX_WAITS_BY_OP:
            lim = MAX_WAITS_BY_OP[opname]
        if si is not None and si.on_wait and len(si.on_wait) > lim:
            waits = list(si.on_wait)
            updates = list(si.on_update or [])
            excess = waits[lim:]
            for i in range(0, len(excess), MAX_WAITS):
                chunk = excess[i : i + MAX_WAITS]
                ev = mybir.InstEventSemaphore(
                    name=self.nc.get_next_instruction_name(),
                    engine=inst.engine,
                    ins=[],
                    outs=[],
                    sync_info=mybir.SyncInfo(on_wait=chunk, on_update=[]),
                )
                orig_add(self, ev)
            inst.sync_info = mybir.SyncInfo(on_wait=waits[:lim], on_update=updates)
        orig_add(self, inst)

    _tile.TileContext._add_instruction = _patched_add
    _tile.TileContext._wait_split_installed = True


# ALiBi decay limits mask tile didx's nonzero band to query columns
# [C_LO, C_HI) (the decay underflows beyond ~110 key-offsets).
C_LO = {0: 0, 1: 0, 2: 128, 3: 256, 4: 384}
C_HI = {0: 128, 1: 256, 2: 384, 3: 512, 4: 512}


def _kbs_for(qb):
    kb_lo = max(0, 4 * qb - 1)
    kb_hi = 4 * qb + 3
    return [
        4 * qb + j - 1 for j in (3, 4, 2, 1, 0) if kb_lo <= 4 * qb + j - 1 <= kb_hi
    ]


def build_nc():
    import concourse.bass as bass
    import concourse.tile as tile
    from concourse import mybir

    _install_drain_patch()
    _install_wait_split_patch()

    F32 = mybir.dt.float32
    BF16 = mybir.dt.bfloat16
    ACTF = mybir.ActivationFunctionType
    MUL = mybir.AluOpType.mult
    ADD = mybir.AluOpType.add

    nc = bass.Bass("TRN2", target_bir_lowering=False, debug=False)

    xt = nc.dram_tensor("xt", [E, S], BF16, kind="ExternalInput")
    wa = nc.dram_tensor("wa", [E, 768], BF16, kind="ExternalInput")
    bqk = nc.dram_tensor("bqk", [P, 4], F32, kind="ExternalInput")
    mt = nc.dram_tensor("mt", [P, NMASK * 512], BF16, kind="ExternalInput")
    wp = nc.dram_tensor("wp", [256, E], BF16, kind="ExternalInput")
    out = nc.dram_tensor("out", [S, E], BF16, kind="ExternalOutput")

    heads = (0, 1, 2, 3)

    with tile.TileContext(nc) as tc, ExitStack() as ctx, nc.allow_low_precision(
        reason="bf16 end-to-end is deliberate; psum accumulation stays fp32"
    ):
        const = ctx.enter_context(tc.tile_pool(name="const", bufs=1))
        bqk_sb = const.tile([P, 4], F32)
        mt_sb = const.tile([P, NMASK * 512], BF16)
        wp_sb = const.tile([P, 2 * 2048], BF16)
        qk_sb = const.tile([P, 4 * 2048], BF16)
        v_sb = const.tile([P, 16 * 260], BF16)
        attn_sb = const.tile([P, 2 * 2048], BF16)

        # ones column of V (per-head col 64) gives the softmax denominator;
        # ones_bf is the K=1 stationary operand of the reciprocal-broadcast
        # matmul (bf16: f32r matmuls run at quarter rate).
        ones128_sb = const.tile([P, 64], F32)
        nc.vector.memset(ones128_sb[:], 1.0)
        ones_bf = const.tile([1, 64], BF16)
        nc.vector.tensor_copy(ones_bf[:], ones128_sb[0:1, :])
        v_ones_view = v_sb[:, :].rearrange("p (g c) -> p g c", c=65)[:, :, 64:65]
        nc.vector.tensor_copy(v_ones_view, ones128_sb[:, :, None])

        # pools that span phase A (sc splicing) and phase B
        exp_pool = ctx.enter_context(tc.tile_pool(name="expp", bufs=8))
        p_pool = ctx.enter_context(tc.tile_pool(name="pp", bufs=78))
        # the sc psum ring lives in whichever phase is emitting chains: a
        # 2-buf ring inside phase A (fits beside the 6 GEMM banks), the
        # 3-buf projection ring in phase B.
        sc_ring = {}
        pts = {}

        def emit_sc_chain(qb, kb, first):
            """score matmul + exp + mask-multiply for one (qb, kb), all heads.
            first=True forces the full 512-column width (the pv start=True
            matmul must cover the whole psum bank row -- zero regions are
            bank-row granular)."""
            didx = kb - 4 * qb + 1  # 0..4
            lo = 0 if first else C_LO[didx]
            hi = C_HI[didx]
            for h in heads:
                hb = (h % 2) * 64
                q_ap = qk_sb[
                    hb : hb + 64,
                    (h // 2) * 2048 + qb * 512 + lo : (h // 2) * 2048 + qb * 512 + hi,
                ]
                k_ap = qk_sb[
                    hb : hb + 64,
                    (2 + h // 2) * 2048 + kb * P : (2 + h // 2) * 2048 + (kb + 1) * P,
                ]
                sc_ps = sc_ring["pool"].tile([P, 512], F32, tag=sc_ring["tag"])
                nc.tensor.matmul(sc_ps[:, lo:hi], k_ap, q_ap, start=True, stop=True)
                e_t = exp_pool.tile([P, 512], BF16)
                nc.scalar.activation(e_t[:, lo:hi], sc_ps[:, lo:hi], ACTF.Exp)
                p_t = p_pool.tile([P, 512], BF16)
                nc.vector.tensor_tensor(
                    p_t[:, lo:hi],
                    e_t[:, lo:hi],
                    mt_sb[:, didx * 512 + lo : didx * 512 + hi],
                    MUL,
                )
                pts[(qb, kb, h)] = (p_t, lo, hi)

        # ---- Phase A: qk^T [512, S] and V [S, 256] in half-passes, with the
        # sc/exp/mask chains of query-block ng-1 spliced into the k-loop ----
        with ExitStack() as actx:
            wa_pool = actx.enter_context(tc.tile_pool(name="wap", bufs=1))
            wa_t = [
                wa_pool.tile([P, 768], BF16, tag=f"wa{k}", name=f"wa{k}")
                for k in range(KE)
            ]
            xt_pool = actx.enter_context(tc.tile_pool(name="xt", bufs=2))
            psA = actx.enter_context(tc.tile_pool(name="psA", bufs=1, space="PSUM"))
            ps_scA = actx.enter_context(
                tc.tile_pool(name="psscA", bufs=2, space="PSUM")
            )
            sc_ring["pool"], sc_ring["tag"] = ps_scA, "sc"

            def load_xt(ng, chunked):
                tiles = []
                for k in range(KE):
                    t = xt_pool.tile([P, 512], BF16, tag=f"xt{k}", name=f"xt{ng}_{k}")
                    if chunked:
                        nc.sync.dma_start(
                            t[:, 0:256],
                            xt[k * P : (k + 1) * P, ng * 512 : ng * 512 + 256],
                        )
                        nc.gpsimd.dma_start(
                            t[:, 256:512],
                            xt[k * P : (k + 1) * P, ng * 512 + 256 : (ng + 1) * 512],
                        )
                    else:
                        eng = nc.sync if k % 2 == 0 else nc.gpsimd
                        eng.dma_start(
                            t[:], xt[k * P : (k + 1) * P, ng * 512 : (ng + 1) * 512]
                        )
                    tiles.append(t)
                return tiles

            # initial loads: wa + xt(ng0) interleaved per k (both chunked) so
            # the first matmuls' tiles head the DMA queues; mt/bqk ride the
            # scalar HW-DGE queue, which is idle during the load.
            xt_tiles = []
            for k in range(KE):
                t = xt_pool.tile([P, 512], BF16, tag=f"xt{k}", name=f"xt0_{k}")
                if k == 0:
                    # k=0 gates the first matmuls: load only what they need
                    # on the sync/gpsimd rings (wa m0/m1 + full xt0 + the v
                    # columns); the half1-only wa block rides the idle
                    # scalar queue.
                    nc.sync.dma_start(
                        wa_t[k][:, 0:256], wa[k * P : (k + 1) * P, 0:256]
                    )
                    nc.gpsimd.dma_start(
                        t[:, 256:512], xt[k * P : (k + 1) * P, 256:512]
                    )
                    nc.sync.dma_start(t[:, 0:256], xt[k * P : (k + 1) * P, 0:256])
                    nc.gpsimd.dma_start(
                        wa_t[k][:, 512:768], wa[k * P : (k + 1) * P, 512:768]
                    )
                    nc.scalar.dma_start(
                        wa_t[k][:, 256:512], wa[k * P : (k + 1) * P, 256:512]
                    )
                else:
                    nc.sync.dma_start(
                        wa_t[k][:, 0:384], wa[k * P : (k + 1) * P, 0:384]
                    )
                    nc.gpsimd.dma_start(
                        wa_t[k][:, 384:768], wa[k * P : (k + 1) * P, 384:768]
                    )
                    nc.sync.dma_start(t[:, 0:256], xt[k * P : (k + 1) * P, 0:256])
                    nc.gpsimd.dma_start(
                        t[:, 256:512], xt[k * P : (k + 1) * P, 256:512]
                    )
                xt_tiles.append(t)
                if k == 0:
                    nc.scalar.dma_start(bqk_sb[:], bqk[:, :])
                if k == 2:
                    nc.scalar.dma_start(mt_sb[:, 0 : 2 * 512], mt[:, 0 : 2 * 512])
                if k == 3:
                    nc.scalar.dma_start(
                        mt_sb[:, 2 * 512 : NMASK * 512], mt[:, 2 * 512 : NMASK * 512]
                    )

            for ng in range(NG):
                # per-(qb=ng-1) sc chains to splice into this ng's k-loops
                chains = []
                if ng >= 1:
                    qbs = ng - 1
                    for idx, kb in enumerate(_kbs_for(qbs)):
                        chains.append((qbs, kb, idx == 0))
                nxt = None
                for half in range(2):
                    qk_ps = [
                        psA.tile([P, 512], F32, tag=f"qk{half}{i}", name=f"qkps{half}{i}")
                        for i in range(2)
                    ]
                    v_ps = [
                        psA.tile([P, 256], F32, tag=f"v{i}", name=f"vps{i}")
                        for i in range(2)
                    ]
                    for k in range(KE):
                        xt_t = xt_tiles[k]
                        for i in range(2):
                            m = half * 2 + i
                            nc.tensor.matmul(
                                qk_ps[i][:],
                                wa_t[k][:, m * P : (m + 1) * P],
                                xt_t[:],
                                start=(k == 0),
                                stop=(k == KE - 1),
                            )
                        for i in range(2):
                            j = half * 2 + i
                            nc.tensor.matmul(
                                v_ps[i][:],
                                xt_t[:, j * P : (j + 1) * P],
                                wa_t[k][:, 512:768],
                                start=(k == 0),
                                stop=(k == KE - 1),
                            )
                        # splice one sc chain after every few k-steps
                        if chains and k % 6 == 2:
                            emit_sc_chain(*chains.pop(0))
                        if half == 0 and ng + 1 < NG and k == KE - 1:
                            nxt = load_xt(ng + 1, chunked=False)
                    # drains: v first (single-buffered tags -- the next half's
                    # v matmuls wait on these), alternating ACT/DVE; qk tags
                    # are double-buffered so their drains overlap freely.
                    for i in range(2):
                        j = half * 2 + i
                        blk = ng * 4 + j
                        vdst = v_sb[:, blk * 260 : blk * 260 + 260].rearrange(
                            "p (h c) -> p h c", c=65
                        )[:, :, 0:64]
                        vsrc = v_ps[i][:].rearrange("p (h c) -> p h c", c=64)
                        if i == 0:
                            nc.vector.tensor_copy(vdst, vsrc)
                        else:
                            nc.scalar.activation(vdst, vsrc, ACTF.Copy)
                    for i in range(2):
                        m = half * 2 + i
                        dst = qk_sb[:, m * 2048 + ng * 512 : m * 2048 + (ng + 1) * 512]
                        if i == 0:
                            nc.scalar.activation(
                                dst, qk_ps[i][:], ACTF.Identity, bias=bqk_sb[:, m : m + 1]
                            )
                        else:
                            nc.vector.tensor_scalar(
                                dst, qk_ps[i][:], bqk_sb[:, m : m + 1], None, op0=ADD
                            )
                    while chains and half == 1:
                        emit_sc_chain(*chains.pop(0))
                if nxt is not None:
                    xt_tiles = nxt

        # ---- Phase B: pv accumulation + normalize + projection (+ the qb3
        # sc chains, which couldn't start until ng3's qk was done) ----
        with ExitStack() as bctx:
            rs_pool = bctx.enter_context(tc.tile_pool(name="rs", bufs=2))
            tmp_pool = bctx.enter_context(tc.tile_pool(name="tmpn", bufs=2))
            bc_pool = bctx.enter_context(tc.tile_pool(name="bcp", bufs=2))
            o_pool = bctx.enter_context(tc.tile_pool(name="op", bufs=6))
            ps_pv = bctx.enter_context(tc.tile_pool(name="pspv", bufs=1, space="PSUM"))
            ps_o = bctx.enter_context(tc.tile_pool(name="pso", bufs=3, space="PSUM"))
            ps_bc = bctx.enter_context(tc.tile_pool(name="psbc", bufs=1, space="PSUM"))
            # qb3's sc chains share the projection psum ring
            sc_ring["pool"], sc_ring["tag"] = ps_o, "oc"
            for kk in range(2):
                nc.gpsimd.dma_start(
                    wp_sb[:, kk * 2048 : (kk + 1) * 2048],
                    wp[kk * P : (kk + 1) * P, :],
                )

            def emit_c_pair(mb, n):
                o_ps = ps_o.tile([P, 512], F32, tag="oc")
                for kk in range(2):
                    nc.tensor.matmul(
                        o_ps[:],
                        attn_sb[:, kk * 2048 + mb * P : kk * 2048 + (mb + 1) * P],
                        wp_sb[:, kk * 2048 + n * 512 : kk * 2048 + (n + 1) * 512],
                        start=(kk == 0),
                        stop=(kk == 1),
                    )
                o_t = o_pool.tile([P, 512], BF16)
                if (mb + n) % 2 == 0:
                    nc.scalar.activation(o_t[:], o_ps[:], ACTF.Copy)
                else:
                    nc.vector.tensor_copy(o_t[:], o_ps[:])
                eng = nc.gpsimd if (mb + n) % 2 == 0 else nc.sync
                eng.dma_start(
                    out[mb * P : (mb + 1) * P, n * 512 : (n + 1) * 512], o_t[:]
                )

            def emit_norm(qb, h, pv_t, inv_sb):
                # broadcast 1/rowsum to 64 partitions via a bf16 K=1 matmul,
                # stage it to SBUF (tensor_tensor may read only one PSUM
                # operand), then multiply straight from the pv psum bank
                # (freeing it for the next qb).
                bc_ps = ps_bc.tile([64, 512], F32, tag="bc")
                nc.tensor.matmul(
                    bc_ps[:],
                    ones_bf[0:1, :],
                    inv_sb[0:1, h * 512 : (h + 1) * 512],
                    start=True,
                    stop=True,
                )
                bc_sb = bc_pool.tile([64, 512], BF16)
                nc.scalar.activation(bc_sb[:], bc_ps[:], ACTF.Copy)
                col0 = (h // 2) * 2048 + qb * 512
                if h % 2 == 0:
                    nc.vector.tensor_tensor(
                        attn_sb[0:64, col0 : col0 + 512],
                        pv_t[0:64, :],
                        bc_sb[:],
                        MUL,
                    )
                else:
                    # DVE lanes can't shift partitions; normalize at
                    # base 0 then DMA-shift to partitions 64..127.
                    tmp_t = tmp_pool.tile([64, 512], BF16)
                    nc.vector.tensor_tensor(
                        tmp_t[:], pv_t[0:64, :], bc_sb[:], MUL
                    )
                    nc.sync.dma_start(
                        attn_sb[64:128, col0 : col0 + 256], tmp_t[:, 0:256]
                    )
                    nc.gpsimd.dma_start(
                        attn_sb[64:128, col0 + 256 : col0 + 512], tmp_t[:, 256:512]
                    )

            # work queue of deferred closures (previous qb's normalization
            # finish + projection pairs), spliced into the step loop as PE
            # filler so the in-order PE queue never head-of-line blocks on
            # the inv round-trip.  pv emission lags the step index by PVOFF
            # so the norms that free each pv bank are always drained (and
            # their bc matmuls queued on PE) before the pv matmul that
            # rewrites the bank -- otherwise the in-order PE queue deadlocks.
            work_q = []
            sc3_todo = [(3, kb, idx == 0) for idx, kb in enumerate(_kbs_for(3))]
            for qb in range(4):
                # per-head rowsums gathered to [32, 16] chunks (32 descriptors
                # per hop instead of 128) for the reciprocal; per head-pair
                # reciprocal so the first pair's normalization starts early.
                rsq_t = rs_pool.tile([32, 64], F32, tag="rsq", name="rsq_t", bufs=2)
                invq_t = rs_pool.tile([32, 64], BF16, tag="invq", name="invq_t", bufs=2)
                inv_sb = rs_pool.tile([1, 4 * 512], BF16, tag="invsb", name="inv_sb", bufs=2)
                rs_row = {
                    h: rs_pool.tile([1, 512], F32, tag=f"rsr{h}", name=f"rs_row{h}", bufs=2)
                    for h in heads
                }
                kbs = _kbs_for(qb)
                pv_ps = {
                    h: ps_pv.tile([65, 512], F32, tag=f"pv{h}", name=f"pvps{h}")
                    for h in heads
                }
                SKEW = 3  # pv offset: prior-qb norms + a couple of proj
                #           pairs drain ahead of the first pv matmul
                nsteps = len(kbs) + SKEW + (7 if qb == 3 else 4)
                for i in range(nsteps):
                    # drain fillers BEFORE pv emission so blocking norms are
                    # queued first
                    if work_q:
                        work_q.pop(0)()
                    # qb3's sc chains (runnable as soon as ng3's qk drained)
                    # fill the phase-B lead-in while qb0's inv round-trip is
                    # in flight.
                    if qb == 0:
                        for _ in range(2):
                            if sc3_todo:
                                emit_sc_chain(*sc3_todo.pop(0))
                    for h in heads:
                        j = i - SKEW - h
                        if 0 <= j < len(kbs):
                            pkb = kbs[j]
                            p_t, plo, phi = pts.pop((qb, pkb, h))
                            nc.tensor.matmul(
                                pv_ps[h][:, plo:phi],
                                v_sb[:, pkb * 260 + 65 * h : pkb * 260 + 65 * h + 65],
                                p_t[:, plo:phi],
                                start=(pkb == kbs[0]),
                                stop=(pkb == kbs[-1]),
                            )
                        elif j == len(kbs):
                            # head h's accumulation is complete: pull the
                            # denominator row out and gather it for the
                            # reciprocal; attn rows stay in psum until
                            # emit_norm.
                            nc.scalar.activation(
                                rs_row[h][:], pv_ps[h][64:65, :], ACTF.Copy
                            )
                            nc.sync.dma_start(
                                rsq_t[:, h * 16 : (h + 1) * 16], rs_row[h][:]
                            )
                            if h % 2 == 1:
                                nc.vector.reciprocal(
                                    invq_t[:, (h - 1) * 16 : (h + 1) * 16],
                                    rsq_t[:, (h - 1) * 16 : (h + 1) * 16],
                                )
                                for hh in (h - 1, h):
                                    nc.sync.dma_start(
                                        inv_sb[0:1, hh * 512 : (hh + 1) * 512],
                                        invq_t[:, hh * 16 : (hh + 1) * 16],
                                    )
                        if qb == 3 and j == len(kbs) + 2:
                            # qb3: normalize inline (the remaining pv matmuls
                            # and fillers overlap the inv round-trip) so the
                            # final flush is projection-only.
                            emit_norm(qb, h, pv_ps[h], inv_sb)
                    if work_q:
                        work_q.pop(0)()
                if qb < 3:
                    for h in heads:
                        work_q.append(
                            (lambda qb=qb, h=h, pv_t=pv_ps[h], inv=inv_sb:
                             emit_norm(qb, h, pv_t, inv))
                        )
                work_q.extend(
                    (lambda mb=mb, n=n: emit_c_pair(mb, n))
                    for mb in range(4 * qb, 4 * qb + 4)
                    for n in range(4)
                )
            while work_q:
                work_q.pop(0)()

    return nc


def build_mask_tiles() -> np.ndarray:
    """mt[p, j*512 + r] = causal/ALiBi multiplicative mask for
    delta0 = (j - 1) * 128, i.e. key c = delta0 + 512*qb... relative offsets:
    t = c - r (tile-local: delta0 + ci - rj); keep exp(m*t) for t <= 0."""
    import ml_dtypes

    ci = np.arange(P)[:, None]
    rj = np.arange(512)[None, :]
    cols = []
    for j in range(NMASK):
        d0 = (j - 1) * 128
        t = d0 + ci - rj
        cols.append(np.where(t <= 0, np.exp(ALIBI_M * t), 0.0))
    arr = np.ascontiguousarray(np.concatenate(cols, axis=1))
    return arr.astype(ml_dtypes.bfloat16)


def make_in_maps(hidden_states, W_attn, b_attn, W_proj):
    import ml_dtypes

    BF = ml_dtypes.bfloat16
    x = np.asarray(hidden_states, dtype=np.float32).reshape(S, E)
    xt = np.ascontiguousarray(x.T).astype(BF)
    Wa = np.asarray(W_attn, dtype=np.float32)
    ba = np.asarray(b_attn, dtype=np.float32)
    Wp = np.asarray(W_proj, dtype=np.float32)
    mt = build_mask_tiles()
    in_maps = []
    for c in range(N_CORES):
        lo, hi = 256 * c, 256 * (c + 1)
        wq = Wa[:, lo:hi] * 0.125
        wk = Wa[:, E + lo : E + hi]
        wv = Wa[:, 2 * E + lo : 2 * E + hi]
        wa_shard = np.ascontiguousarray(
            np.concatenate([wq, wk, wv], axis=1)
        ).astype(BF)
        bqk = np.concatenate([ba[lo:hi] * 0.125, ba[E + lo : E + hi]])
        bqk_mat = np.ascontiguousarray(bqk.reshape(4, P).T)
        wp_shard = np.ascontiguousarray(Wp[lo:hi, :]).astype(BF)
        in_maps.append(
            {"xt": xt, "wa": wa_shard, "bqk": bqk_mat, "mt": mt, "wp": wp_shard}
        )
    return in_maps


_NC_CACHE = {}


def kernel(hidden_states, W_attn, b_attn, W_proj, b_proj):
    from concourse.bass_utils import run_bass_kernel_spmd

    if "nc" not in _NC_CACHE:
        _NC_CACHE["nc"] = build_nc()
    nc = _NC_CACHE["nc"]

    in_maps = make_in_maps(hidden_states, W_attn, b_attn, W_proj)
    res = run_bass_kernel_spmd(nc, in_maps, core_ids=list(range(N_CORES)))

    out = np.zeros((S, E), dtype=np.float32)
    for c in range(N_CORES):
        out += np.asarray(res.results[c]["out"], dtype=np.float32)
    ba = np.asarray(b_attn, dtype=np.float32)
    bp = np.asarray(b_proj, dtype=np.float32)
    Wp = np.asarray(W_proj, dtype=np.float32)
    # v-bias passes through softmax linearly (rows sum to 1): fold on host.
    out += ba[2 * E :] @ Wp + bp
    return out.reshape(1, S, E).astype(np.float32)
